# revision 6
# baseline (speedup 1.0000x reference)
"""Trainium2 Bass kernel for nn_DSRLossStateless (DSR loss, stateless).

loss = -sum_t(D_t)/B where D_t comes from an eta-EMA pair (A,B) over
portfolio returns R_t = sum_a w[t,a]*nr[t,a].

Strategy (8 cores, batch-sharded):
  - Each core owns 250k consecutive rows, laid out as SBUF partitions
    1..125 x 2000 columns (time-major within a partition). Partition 0
    holds the 2000 rows preceding the core's span (synthetic for core 0),
    which supplies the scan carry for partition 1.
  - Scans run in a-units (a=A/eta, b=B/eta): a_t = c*a_{t-1} + R_t,
    b_t = c*b_{t-1} + R_t^2; the final host scale becomes sqrt(eta) and
    the eps clamp 1e-6.
  - Bulk: 16 x 1MB DMA chunks per tensor (two via the gpsimd SWDGE queue
    as a third ring); per 125-col chunk: product on GPSIMD, rowsum-of-16
    + chained local scans on DVE, square on ACT.
  - D-chain  q = [0.5a(R^2+eta*b) - b*R] * max(b-eta*a^2,1e-6)^-1.5 runs
    in 4 col-blocks; var^-1.5 = exp(-1.5*ln(var)) on ACT. Blocks t>=1000
    skip the partition carry (decay c^1000~4e-5 is negligible): block j2
    overlaps the bulk, j3 follows the last scan; blocks t<1000 apply
    carry = previous partition's local scan final via fused STT after a
    tiny column->row DMA hop, stage-interleaved across DVE/ACT/GPSIMD.
  - Host: loss = sqrt(eta) * sum(partials) / B.
"""

import sys

sys.path.insert(0, "/opt/trn_rl_repo")

import numpy as np

import concourse.bass as bass
import concourse.bacc as bacc
import concourse.tile as tile
from concourse import mybir
from concourse.bass_utils import run_bass_kernel_spmd
from contextlib import ExitStack

F32 = mybir.dt.float32
NF32 = np.float32

N_CORES = 8
NA = 16            # assets (inner dim)
KP = 126           # SBUF partitions used (0 = prepend/carry-feeder)
L = 2000           # columns (time steps per partition)
LE = L + 1         # local-scan buffer width (col 0 = zero carry)
OWN = (KP - 1) * L      # rows owned per core = 250000
B_TOTAL = N_CORES * OWN # 2000000
CH = 16            # compute chunks = DMA chunks
KC = L // CH       # 125 cols per chunk
FW = KC * NA       # 2000 f32 per partition per chunk
SWDGE_KS = (4, 5)  # chunks routed via gpsimd SWDGE (3rd DMA ring)
HB = 500           # D-chain col-block width
ETA = 0.01
EPS = 1e-8
EPSU = EPS / ETA   # eps clamp in a-units = 1e-6
SQETA = float(np.sqrt(NF32(ETA)))
CDEC = NF32(1.0 - ETA)  # 0.99
SQE = float(np.sqrt(ETA))  # fold eta into ACT square scale

AL = mybir.AluOpType
AF = mybir.ActivationFunctionType
AX = mybir.AxisListType

_PROGRAM = None


def _build_program():
    nc = bacc.Bacc("TRN2", target_bir_lowering=False, debug=False)

    w_ap = nc.dram_tensor("w", [KP * L, NA], F32, kind="ExternalInput").ap()
    nr_ap = nc.dram_tensor("nr", [KP * L, NA], F32, kind="ExternalInput").ap()
    out_ap = nc.dram_tensor("out", [1, 1], F32, kind="ExternalOutput").ap()

    # geo_c[t] = c^t (carry decay for the correction pass)
    geoc_np = (CDEC ** np.arange(L).astype(NF32)).astype(NF32)
    geoc_dram = nc.inline_tensor(
        np.ascontiguousarray(np.broadcast_to(geoc_np, (KP, L))), name="geoc"
    )

    w_v = w_ap.rearrange("(p t) a -> p (t a)", p=KP)
    nr_v = nr_ap.rearrange("(p t) a -> p (t a)", p=KP)

    with tile.TileContext(nc) as tc, ExitStack() as ctx:
        pers = ctx.enter_context(tc.tile_pool(name="pers", bufs=1))
        loadp = ctx.enter_context(tc.tile_pool(name="load", bufs=5))
        tmpp = ctx.enter_context(tc.tile_pool(name="tmp", bufs=4))

        R = pers.tile([KP, L], F32, tag="R")
        R2 = pers.tile([KP, L], F32, tag="R2")
        Aloc = pers.tile([KP, LE], F32, tag="Aloc")
        Bloc = pers.tile([KP, LE], F32, tag="Bloc")
        geoc = pers.tile([KP, L], F32, tag="geoc")
        cvec = pers.tile([KP, KC], F32, tag="cvec")
        initA = pers.tile([KP, 1], F32, tag="initA")
        initB = pers.tile([KP, 1], F32, tag="initB")
        qsum = pers.tile([KP, 4], F32, tag="qsum")
        qred = pers.tile([KP, 1], F32, tag="qred")
        qrow = pers.tile([1, KP - 1], F32, tag="qrow")
        qtot = pers.tile([1, 1], F32, tag="qtot")

        # pre-issue the first two chunk DMAs before anything else, plus the
        # SWDGE-routed chunks (their Q7 descriptor-gen must precede the muls
        # in the gpsimd queue)
        lds = {}
        for k in (0, 1) + SWDGE_KS:
            wt = loadp.tile([KP, FW], F32, tag="wt", name=f"wt{k}")
            rt = loadp.tile([KP, FW], F32, tag="rt", name=f"rt{k}")
            if k in SWDGE_KS:
                nc.gpsimd.dma_start(wt[:], w_v[:, k * FW:(k + 1) * FW])
                nc.gpsimd.dma_start(rt[:], nr_v[:, k * FW:(k + 1) * FW])
            else:
                nc.sync.dma_start(wt[:], w_v[:, k * FW:(k + 1) * FW])
                nc.scalar.dma_start(rt[:], nr_v[:, k * FW:(k + 1) * FW])
            lds[k] = (wt, rt)

        # constants / scan seeds; pin ACT tables (sqrt, square, ln, exp)
        nc.vector.memset(qtot[0:1, 0:1], 1.0)
        nc.scalar.sqrt(qtot[0:1, 0:1], qtot[0:1, 0:1])
        nc.scalar.square(qtot[0:1, 0:1], qtot[0:1, 0:1])
        nc.scalar.activation(qtot[0:1, 0:1], qtot[0:1, 0:1], AF.Ln)
        nc.scalar.activation(qtot[0:1, 0:1], qtot[0:1, 0:1], AF.Exp)
        nc.vector.memset(cvec[:, :], float(CDEC))
        nc.vector.memset(Aloc[:, 0:1], 0.0)
        nc.vector.memset(Bloc[:, 0:1], 0.0)
        nc.vector.memset(initA[:, 0:1], 0.0)
        nc.vector.memset(initB[:, 0:1], 0.0)

        def dchain(j, cs, fix):
            """Straight-line q-block over cols [cs, cs+HB) (used for j2/j3)."""
            ce = cs + HB
            ap_, bp_ = Aloc[:, cs:ce], Bloc[:, cs:ce]
            assert not fix
            a2e = tmpp.tile([KP, HB], F32, tag="a2e", name=f"a2e{j}")
            nc.scalar.activation(a2e[:, :], ap_, AF.Square, scale=SQE)
            var = tmpp.tile([KP, HB], F32, tag="var", name=f"var{j}")
            nc.gpsimd.tensor_sub(var[:, :], bp_, a2e[:, :])
            nc.scalar.mul(a2e[:, :], bp_, ETA)              # be (reuse a2e)
            t3 = tmpp.tile([KP, HB], F32, tag="t3", name=f"t3{j}")
            nc.gpsimd.tensor_mul(t3[:, :], bp_, R[:, cs:ce])
            nc.vector.tensor_scalar_max(var[:, :], var[:, :], EPSU)
            lnt = tmpp.tile([KP, HB], F32, tag="lnt", name=f"lnt{j}")
            nc.scalar.activation(lnt[:, :], var[:, :], AF.Ln)
            rr = tmpp.tile([KP, HB], F32, tag="rr", name=f"rr{j}")
            nc.scalar.activation(rr[:, :], lnt[:, :], AF.Exp, scale=-1.5)
            t1 = tmpp.tile([KP, HB], F32, tag="t1", name=f"t1{j}")
            nc.gpsimd.tensor_add(t1[:, :], a2e[:, :], R2[:, cs:ce])
            n0 = tmpp.tile([KP, HB], F32, tag="n0", name=f"n0{j}")
            nc.vector.scalar_tensor_tensor(
                out=n0[:, :], in0=ap_, scalar=0.5, in1=t1[:, :],
                op0=AL.mult, op1=AL.mult,
            )
            nc.vector.tensor_sub(var[:, :], n0[:, :], t3[:, :])  # n1 (reuse var)
            nc.vector.scalar_tensor_tensor(                      # q + row-accum
                out=n0[:, :], in0=var[:, :], scalar=1.0, in1=rr[:, :],
                op0=AL.mult, op1=AL.mult, accum_out=qsum[:, j:j + 1],
            )

        # ---- bulk: chunked DMA + product/reduce/scan pipeline ----
        for k in range(CH):
            if k in lds:
                wt, rt = lds[k]
            else:
                wt = loadp.tile([KP, FW], F32, tag="wt", name=f"wt{k}")
                rt = loadp.tile([KP, FW], F32, tag="rt", name=f"rt{k}")
                nc.sync.dma_start(wt[:], w_v[:, k * FW:(k + 1) * FW])
                nc.scalar.dma_start(rt[:], nr_v[:, k * FW:(k + 1) * FW])
            ks = slice(k * KC, (k + 1) * KC)
            nc.gpsimd.tensor_mul(wt[:], wt[:], rt[:])
            nc.vector.reduce_sum(
                R[:, ks], wt[:].rearrange("p (t a) -> p t a", a=NA), axis=AX.X
            )
            nc.scalar.square(R2[:, ks], R[:, ks])
            # chained zero-carry local scans: state = c*state + x
            nc.vector.tensor_tensor_scan(
                out=Aloc[:, 1 + k * KC:1 + (k + 1) * KC], data0=cvec[:, :],
                data1=R[:, ks], initial=Aloc[:, k * KC:k * KC + 1],
                op0=AL.mult, op1=AL.add,
            )
            nc.vector.tensor_tensor_scan(
                out=Bloc[:, 1 + k * KC:1 + (k + 1) * KC], data0=cvec[:, :],
                data1=R2[:, ks], initial=Bloc[:, k * KC:k * KC + 1],
                op0=AL.mult, op1=AL.add,
            )
            if k == 11:
                dchain(2, 1000, False)   # overlaps remaining bulk
            if k == CH - 1:
                # geoc halves keep the two HWDGE queues balanced;
                # needed only by the j0/j1 fix STTs
                nc.sync.dma_start(geoc[:, 0:1000], geoc_dram.ap()[:, 0:1000])
                nc.scalar.dma_start(geoc[:, 1000:L], geoc_dram.ap()[:, 1000:L])

        # ---- tail ----
        # carries: previous partition's local final (c^2000 kills older)
        nc.sync.dma_start(initA[1:KP, 0:1], Aloc[0:KP - 1, L:LE])
        nc.scalar.dma_start(initB[1:KP, 0:1], Bloc[0:KP - 1, L:LE])

        dchain(3, 1500, False)

        # j0/j1 fix blocks, stage-interleaved across engines.
        T = {}
        for j, cs in ((0, 0), (1, 500)):
            for tag in ("aprev", "bprev", "a2e", "var", "lnt", "rr", "t3", "n0"):
                T[tag, j] = tmpp.tile(
                    [KP, HB], F32, tag=tag, name=f"{tag}_{j}",
                    bufs=2 if tag in ("aprev", "bprev") else 4,
                )

        def fix(j, cs):
            nc.vector.scalar_tensor_tensor(
                out=T["aprev", j][:, :], in0=geoc[:, cs:cs + HB],
                scalar=initA[:, 0:1], in1=Aloc[:, cs:cs + HB],
                op0=AL.mult, op1=AL.add,
            )
            nc.vector.scalar_tensor_tensor(
                out=T["bprev", j][:, :], in0=geoc[:, cs:cs + HB],
                scalar=initB[:, 0:1], in1=Bloc[:, cs:cs + HB],
                op0=AL.mult, op1=AL.add,
            )

        def a2e(j):
            nc.scalar.activation(
                T["a2e", j][:, :], T["aprev", j][:, :], AF.Square, scale=SQE)

        def var(j):
            nc.gpsimd.tensor_sub(
                T["var", j][:, :], T["bprev", j][:, :], T["a2e", j][:, :])

        def be(j):  # eta*b, reusing the a2e tile
            nc.scalar.mul(T["a2e", j][:, :], T["bprev", j][:, :], ETA)

        def t3(j):
            nc.gpsimd.tensor_mul(
                T["t3", j][:, :], T["bprev", j][:, :], R[:, 500 * j:500 * j + HB])

        def vmax(j):
            nc.vector.tensor_scalar_max(
                T["var", j][:, :], T["var", j][:, :], EPSU)

        def lnj(j):
            nc.scalar.activation(T["lnt", j][:, :], T["var", j][:, :], AF.Ln)

        def expj(j):
            nc.scalar.activation(
                T["rr", j][:, :], T["lnt", j][:, :], AF.Exp, scale=-1.5)

        def t1(j):  # on DVE: t1 = be + R2, into the be/a2e tile
            nc.vector.tensor_add(
                T["a2e", j][:, :], T["a2e", j][:, :], R2[:, 500 * j:500 * j + HB])

        def n0(j):
            nc.vector.scalar_tensor_tensor(
                out=T["n0", j][:, :], in0=T["aprev", j][:, :], scalar=0.5,
                in1=T["a2e", j][:, :], op0=AL.mult, op1=AL.mult,
            )

        def n1(j):  # into the var tile
            nc.vector.tensor_sub(
                T["var", j][:, :], T["n0", j][:, :], T["t3", j][:, :])

        def q(j):
            nc.vector.scalar_tensor_tensor(
                out=T["n0", j][:, :], in0=T["var", j][:, :], scalar=1.0,
                in1=T["rr", j][:, :], op0=AL.mult, op1=AL.mult,
                accum_out=qsum[:, j:j + 1],
            )

        fix(0, 0)
        a2e(0); var(0); be(0); t3(0)
        fix(1, 500)
        vmax(0); lnj(0); expj(0)
        a2e(1); var(1); be(1); t3(1)
        t1(0); n0(0)
        vmax(1); lnj(1); expj(1)
        n1(0); q(0)
        t1(1); n0(1); n1(1); q(1)

        # partition reduce: 4 block partials -> 1, flatten, reduce, store
        nc.vector.reduce_sum(qred[:, 0:1], qsum[:, 0:4], axis=AX.X)
        nc.sync.dma_start(qrow[0:1, 0:KP - 1], qred[1:KP, 0:1])
        nc.vector.reduce_sum(qtot[0:1, 0:1], qrow[0:1, 0:KP - 1], axis=AX.X)
        nc.sync.dma_start(out_ap[0:1, 0:1], qtot[0:1, 0:1])

    nc.compile()
    return nc


def _get_program():
    global _PROGRAM
    if _PROGRAM is None:
        _PROGRAM = _build_program()
    return _PROGRAM


def _core0_prepend():
    """2000 synthetic rows encoding the global init (a,b)=(0,EPS/ETA).

    All-zero rows leave the scan at (0,0); the last two rows carry returns
    r1, r2 with r2 = -fl(c*r1) so the a-scan cancels to ~0, while
    c*r1^2 + r2^2 ~ EPS/ETA supplies the b carry.
    """
    w = np.zeros((L, NA), NF32)
    nr = np.zeros((L, NA), NF32)
    c = CDEC
    r1 = NF32(np.sqrt(EPS / (ETA * (float(c) + float(c) ** 2))))
    r2 = NF32(-(c * r1))
    w[L - 2, 0] = NF32(1.0)
    nr[L - 2, 0] = r1
    w[L - 1, 0] = NF32(1.0)
    nr[L - 1, 0] = r2
    return w, nr


def _make_in_maps(weights, nr):
    weights = np.ascontiguousarray(weights, dtype=NF32)
    nr = np.ascontiguousarray(nr, dtype=NF32)
    pre_w, pre_nr = _core0_prepend()
    in_maps = []
    for m in range(N_CORES):
        s = m * OWN
        if m == 0:
            wm = np.concatenate([pre_w, weights[:OWN]])
            rm = np.concatenate([pre_nr, nr[:OWN]])
        else:
            wm = weights[s - L:s + OWN]
            rm = nr[s - L:s + OWN]
        in_maps.append({"w": wm, "nr": rm})
    return in_maps


def _run(in_maps, **kwargs):
    nc = _get_program()
    return run_bass_kernel_spmd(nc, in_maps, core_ids=list(range(N_CORES)), **kwargs)


def kernel(weights, next_returns):
    in_maps = _make_in_maps(weights, next_returns)
    res = _run(in_maps)
    total = np.sum(
        np.array([res.results[m]["out"][0, 0] for m in range(N_CORES)], NF32),
        dtype=NF32,
    )
    return NF32(NF32(SQETA) * total / NF32(B_TOTAL))


# revision 12
# speedup vs baseline: 1.1708x; 1.1708x over previous
"""Trainium2 Bass kernel for nn_DSRLossStateless (DSR loss, stateless).

loss = -sum_t(D_t)/B where D_t comes from an eta-EMA pair (A,B) over
portfolio returns R_t = sum_a w[t,a]*nr[t,a].

Strategy (8 cores, batch-sharded):
  - Each core owns 250k consecutive rows, laid out as SBUF partitions
    1..125 x 2000 columns (time-major within a partition). Partition 0
    holds the 2000 rows preceding the core's span (synthetic for core 0),
    which supplies the scan carry for partition 1.
  - Scans run in a-units (a=A/eta, b=B/eta): a_t = c*a_{t-1} + R_t,
    b_t = c*b_{t-1} + R_t^2; the final host scale becomes sqrt(eta) and
    the eps clamp 1e-6.
  - Bulk: 16 x 1MB DMA chunks per tensor (two via the gpsimd SWDGE queue
    as a third ring); per 125-col chunk: product on GPSIMD, rowsum-of-16
    + chained local scans on DVE, square on ACT.
  - D-chain  q = [0.5a(R^2+eta*b) - b*R] * max(b-eta*a^2,1e-6)^-1.5 runs
    in 4 col-blocks; var^-1.5 = exp(-1.5*ln(var)) on ACT. Blocks t>=1000
    skip the partition carry (decay c^1000~4e-5 is negligible): block j2
    overlaps the bulk, j3 follows the last scan; blocks t<1000 apply
    carry = previous partition's local scan final via fused STT after a
    tiny column->row DMA hop, stage-interleaved across DVE/ACT/GPSIMD.
  - Host: loss = sqrt(eta) * sum(partials) / B.
"""

import sys

sys.path.insert(0, "/opt/trn_rl_repo")

import numpy as np

import concourse.bass as bass
import concourse.bacc as bacc
import concourse.tile as tile
from concourse import mybir
from concourse.bass_utils import run_bass_kernel_spmd
from contextlib import ExitStack

F32 = mybir.dt.float32
NF32 = np.float32

N_CORES = 8
NA = 16            # assets (inner dim)
KP = 126           # SBUF partitions used (0 = prepend/carry-feeder)
L = 2000           # columns (time steps per partition)
LE = L + 1         # local-scan buffer width (col 0 = zero carry)
OWN = (KP - 1) * L      # rows owned per core = 250000
B_TOTAL = N_CORES * OWN # 2000000
CH = 16            # compute chunks = DMA chunks
KC = L // CH       # 125 cols per chunk
FW = KC * NA       # 2000 f32 per partition per chunk
SWDGE_KS = (4, 5)  # chunks routed via gpsimd SWDGE (3rd DMA ring)
HB = 500           # D-chain col-block width
ETA = 0.01
EPS = 1e-8
EPSU = EPS / ETA   # eps clamp in a-units = 1e-6
SQETA = float(np.sqrt(NF32(ETA)))
CDEC = NF32(1.0 - ETA)  # 0.99
SQE = float(np.sqrt(ETA))  # fold eta into ACT square scale

AL = mybir.AluOpType
AF = mybir.ActivationFunctionType
AX = mybir.AxisListType

_PROGRAM = None


def _build_program():
    nc = bacc.Bacc("TRN2", target_bir_lowering=False, debug=False)

    w_ap = nc.dram_tensor("w", [KP * L, NA], F32, kind="ExternalInput").ap()
    nr_ap = nc.dram_tensor("nr", [KP * L, NA], F32, kind="ExternalInput").ap()
    out_ap = nc.dram_tensor("out", [1, 1], F32, kind="ExternalOutput").ap()

    # geo_c[t] = c^t (carry decay for the correction pass)
    geoc_np = (CDEC ** np.arange(L).astype(NF32)).astype(NF32)
    geoc_dram = nc.inline_tensor(
        np.ascontiguousarray(np.broadcast_to(geoc_np, (KP, L))), name="geoc"
    )

    w_v = w_ap.rearrange("(p t) a -> p (t a)", p=KP)
    nr_v = nr_ap.rearrange("(p t) a -> p (t a)", p=KP)

    with tile.TileContext(nc) as tc, ExitStack() as ctx:
        pers = ctx.enter_context(tc.tile_pool(name="pers", bufs=1))
        loadp = ctx.enter_context(tc.tile_pool(name="load", bufs=5))
        tmpp = ctx.enter_context(tc.tile_pool(name="tmp", bufs=4))

        R = pers.tile([KP, L], F32, tag="R")
        R2 = pers.tile([KP, L], F32, tag="R2")
        Aloc = pers.tile([KP, LE], F32, tag="Aloc")
        Bloc = pers.tile([KP, LE], F32, tag="Bloc")
        geoc = pers.tile([KP, L], F32, tag="geoc")
        cvec = pers.tile([KP, KC], F32, tag="cvec")
        initA = pers.tile([KP, 1], F32, tag="initA")
        initB = pers.tile([KP, 1], F32, tag="initB")
        qsum = pers.tile([KP, 4], F32, tag="qsum")
        qred = pers.tile([KP, 1], F32, tag="qred")
        qrow = pers.tile([1, KP - 1], F32, tag="qrow")
        qtot = pers.tile([1, 1], F32, tag="qtot")

        # pre-issue the first two chunk DMAs before anything else
        lds = {}
        for k in (0, 1):
            wt = loadp.tile([KP, FW], F32, tag="wt", name=f"wt{k}")
            rt = loadp.tile([KP, FW], F32, tag="rt", name=f"rt{k}")
            nc.sync.dma_start(wt[:], w_v[:, k * FW:(k + 1) * FW])
            nc.scalar.dma_start(rt[:], nr_v[:, k * FW:(k + 1) * FW])
            lds[k] = (wt, rt)

        # constants / scan seeds; pin the only ACT tables used
        # (square in bulk, ln+exp in the D-chain) to avoid table thrash
        nc.vector.memset(qtot[0:1, 0:1], 1.0)
        nc.scalar.square(qtot[0:1, 0:1], qtot[0:1, 0:1])
        nc.scalar.activation(qtot[0:1, 0:1], qtot[0:1, 0:1], AF.Ln)
        nc.scalar.activation(qtot[0:1, 0:1], qtot[0:1, 0:1], AF.Exp)
        nc.vector.memset(cvec[:, :], float(CDEC))
        nc.vector.memset(Aloc[:, 0:1], 0.0)
        nc.vector.memset(Bloc[:, 0:1], 0.0)
        nc.vector.memset(initA[:, 0:1], 0.0)
        nc.vector.memset(initB[:, 0:1], 0.0)

        def dchain(j, cs):
            """Straight-line q-block over cols [cs, cs+HB), no carry fix
            (used for j2/j3 where the correction decay c^1000 is negligible).
            var^-1.5 = exp(-1.5*ln(var)); var>0 holds structurally except on
            the excluded synthetic partition 0."""
            ce = cs + HB
            ap_, bp_ = Aloc[:, cs:ce], Bloc[:, cs:ce]
            a2e = tmpp.tile([KP, HB], F32, tag="a2e", name=f"a2e{j}")
            nc.vector.scalar_tensor_tensor(              # eta*a^2
                out=a2e[:, :], in0=ap_, scalar=ETA, in1=ap_,
                op0=AL.mult, op1=AL.mult,
            )
            var = tmpp.tile([KP, HB], F32, tag="var", name=f"var{j}")
            nc.gpsimd.tensor_sub(var[:, :], bp_, a2e[:, :])
            t3 = tmpp.tile([KP, HB], F32, tag="t3", name=f"t3{j}")
            nc.gpsimd.tensor_mul(t3[:, :], bp_, R[:, cs:ce])
            lnt = tmpp.tile([KP, HB], F32, tag="lnt", name=f"lnt{j}")
            nc.scalar.activation(lnt[:, :], var[:, :], AF.Ln)
            rr = tmpp.tile([KP, HB], F32, tag="rr", name=f"rr{j}")
            nc.scalar.activation(rr[:, :], lnt[:, :], AF.Exp, scale=-1.5)
            t1 = tmpp.tile([KP, HB], F32, tag="t1", name=f"t1{j}")
            nc.vector.scalar_tensor_tensor(              # R^2 + eta*b
                out=t1[:, :], in0=bp_, scalar=ETA, in1=R2[:, cs:ce],
                op0=AL.mult, op1=AL.add,
            )
            n0 = tmpp.tile([KP, HB], F32, tag="n0", name=f"n0{j}")
            nc.vector.scalar_tensor_tensor(
                out=n0[:, :], in0=ap_, scalar=0.5, in1=t1[:, :],
                op0=AL.mult, op1=AL.mult,
            )
            nc.vector.tensor_sub(var[:, :], n0[:, :], t3[:, :])  # n1 (reuse var)
            nc.vector.scalar_tensor_tensor(                      # q + row-accum
                out=n0[:, :], in0=var[:, :], scalar=1.0, in1=rr[:, :],
                op0=AL.mult, op1=AL.mult, accum_out=qsum[:, j:j + 1],
            )

        # ---- bulk: chunked DMA + product/reduce/scan pipeline ----
        for k in range(CH):
            if k in lds:
                wt, rt = lds[k]
            else:
                wt = loadp.tile([KP, FW], F32, tag="wt", name=f"wt{k}")
                rt = loadp.tile([KP, FW], F32, tag="rt", name=f"rt{k}")
                if k == CH - 1:
                    # split the last chunk's DMA so its mini-muls can
                    # start before the full MB lands
                    HF = FW // 2
                    c0 = k * FW
                    nc.sync.dma_start(wt[:, 0:HF], w_v[:, c0:c0 + HF])
                    nc.scalar.dma_start(rt[:, 0:HF], nr_v[:, c0:c0 + HF])
                    nc.sync.dma_start(wt[:, HF:FW], w_v[:, c0 + HF:c0 + FW])
                    nc.scalar.dma_start(rt[:, HF:FW], nr_v[:, c0 + HF:c0 + FW])
                    # geoc halves last: off the critical path (needed only
                    # by the j0/j1 fix STTs), queues stay balanced
                    nc.sync.dma_start(geoc[:, 0:1000], geoc_dram.ap()[:, 0:1000])
                    nc.scalar.dma_start(geoc[:, 1000:L], geoc_dram.ap()[:, 1000:L])
                else:
                    nc.sync.dma_start(wt[:], w_v[:, k * FW:(k + 1) * FW])
                    nc.scalar.dma_start(rt[:], nr_v[:, k * FW:(k + 1) * FW])
            # the straggler chunks: k14's product on DVE (runs while k15
            # streams), k15's as two gpsimd mini-products
            subs = [(0, KC)]
            eng = nc.gpsimd
            if k == CH - 2:
                eng = nc.vector
            elif k == CH - 1:
                subs = [(0, KC // 2), (KC // 2, KC)]
            for lo, hi in subs:
                wsl = wt[:, lo * NA:hi * NA]
                eng.tensor_mul(wsl, wsl, rt[:, lo * NA:hi * NA])
                ks = slice(k * KC + lo, k * KC + hi)
                nc.vector.reduce_sum(
                    R[:, ks], wsl.rearrange("p (t a) -> p t a", a=NA), axis=AX.X
                )
                nc.scalar.square(R2[:, ks], R[:, ks])
                # chained zero-carry local scans: state = c*state + x
                nc.vector.tensor_tensor_scan(
                    out=Aloc[:, 1 + k * KC + lo:1 + k * KC + hi],
                    data0=cvec[:, 0:hi - lo], data1=R[:, ks],
                    initial=Aloc[:, k * KC + lo:k * KC + lo + 1],
                    op0=AL.mult, op1=AL.add,
                )
                nc.vector.tensor_tensor_scan(
                    out=Bloc[:, 1 + k * KC + lo:1 + k * KC + hi],
                    data0=cvec[:, 0:hi - lo], data1=R2[:, ks],
                    initial=Bloc[:, k * KC + lo:k * KC + lo + 1],
                    op0=AL.mult, op1=AL.add,
                )
            if k == 11:
                dchain(2, 1000)   # overlaps remaining bulk

        # ---- tail ----
        # carries: previous partition's local final (c^2000 kills older)
        nc.sync.dma_start(initA[1:KP, 0:1], Aloc[0:KP - 1, L:LE])
        nc.scalar.dma_start(initB[1:KP, 0:1], Bloc[0:KP - 1, L:LE])

        # blocks j3 (nofix, cols 1500:2000) + j0/j1 (fix, cols 0:1000),
        # stage-interleaved across engines (per-engine queues are in-order)
        T = {}
        for j in (0, 1, 3):
            tags = ("a2e", "var", "lnt", "rr", "t3", "t1", "n0")
            if j != 3:
                tags += ("aprev", "bprev")
            for tag in tags:
                T[tag, j] = tmpp.tile(
                    [KP, HB], F32, tag=tag, name=f"{tag}_{j}",
                    bufs=2 if tag in ("aprev", "bprev") else 4,
                )

        def srcs(j):
            cs = {0: 0, 1: 500, 3: 1500}[j]
            if j == 3:
                return Aloc[:, cs:cs + HB], Bloc[:, cs:cs + HB], cs
            return T["aprev", j][:, :], T["bprev", j][:, :], cs

        def fix(j, cs):
            nc.vector.scalar_tensor_tensor(
                out=T["aprev", j][:, :], in0=geoc[:, cs:cs + HB],
                scalar=initA[:, 0:1], in1=Aloc[:, cs:cs + HB],
                op0=AL.mult, op1=AL.add,
            )
            nc.vector.scalar_tensor_tensor(
                out=T["bprev", j][:, :], in0=geoc[:, cs:cs + HB],
                scalar=initB[:, 0:1], in1=Bloc[:, cs:cs + HB],
                op0=AL.mult, op1=AL.add,
            )

        def a2e(j):  # eta*a^2 on DVE
            ap_, _, _ = srcs(j)
            nc.vector.scalar_tensor_tensor(
                out=T["a2e", j][:, :], in0=ap_, scalar=ETA, in1=ap_,
                op0=AL.mult, op1=AL.mult,
            )

        def var(j):
            _, bp_, _ = srcs(j)
            nc.gpsimd.tensor_sub(T["var", j][:, :], bp_, T["a2e", j][:, :])

        def t3(j):
            _, bp_, cs = srcs(j)
            nc.gpsimd.tensor_mul(T["t3", j][:, :], bp_, R[:, cs:cs + HB])

        def lnj(j):
            nc.scalar.activation(T["lnt", j][:, :], T["var", j][:, :], AF.Ln)

        def expj(j):
            nc.scalar.activation(
                T["rr", j][:, :], T["lnt", j][:, :], AF.Exp, scale=-1.5)

        def t1(j):  # R^2 + eta*b on DVE
            _, bp_, cs = srcs(j)
            nc.vector.scalar_tensor_tensor(
                out=T["t1", j][:, :], in0=bp_, scalar=ETA,
                in1=R2[:, cs:cs + HB], op0=AL.mult, op1=AL.add,
            )

        def n0(j):
            ap_, _, _ = srcs(j)
            nc.vector.scalar_tensor_tensor(
                out=T["n0", j][:, :], in0=ap_, scalar=0.5,
                in1=T["t1", j][:, :], op0=AL.mult, op1=AL.mult,
            )

        def n1(j):  # into the var tile
            nc.vector.tensor_sub(
                T["var", j][:, :], T["n0", j][:, :], T["t3", j][:, :])

        def q(j):
            nc.vector.scalar_tensor_tensor(
                out=T["n0", j][:, :], in0=T["var", j][:, :], scalar=1.0,
                in1=T["rr", j][:, :], op0=AL.mult, op1=AL.mult,
                accum_out=qsum[:, j:j + 1],
            )

        a2e(3); var(3); t3(3); lnj(3); expj(3)
        fix(0, 0)
        fix(1, 500)
        t1(3); n0(3)
        a2e(0); var(0); t3(0); lnj(0); expj(0)
        n1(3); q(3)
        a2e(1); var(1); t3(1); lnj(1); expj(1)
        t1(0); n0(0); n1(0); q(0)
        t1(1); n0(1); n1(1); q(1)

        # partition reduce: 4 block partials -> 1, flatten, reduce, store
        nc.vector.reduce_sum(qred[:, 0:1], qsum[:, 0:4], axis=AX.X)
        nc.sync.dma_start(qrow[0:1, 0:KP - 1], qred[1:KP, 0:1])
        nc.vector.reduce_sum(qtot[0:1, 0:1], qrow[0:1, 0:KP - 1], axis=AX.X)
        nc.sync.dma_start(out_ap[0:1, 0:1], qtot[0:1, 0:1])

    nc.compile()
    return nc


def _get_program():
    global _PROGRAM
    if _PROGRAM is None:
        _PROGRAM = _build_program()
    return _PROGRAM


def _core0_prepend():
    """2000 synthetic rows encoding the global init (a,b)=(0,EPS/ETA).

    All-zero rows leave the scan at (0,0); the last two rows carry returns
    r1, r2 with r2 = -fl(c*r1) so the a-scan cancels to ~0, while
    c*r1^2 + r2^2 ~ EPS/ETA supplies the b carry.
    """
    w = np.zeros((L, NA), NF32)
    nr = np.zeros((L, NA), NF32)
    c = CDEC
    r1 = NF32(np.sqrt(EPS / (ETA * (float(c) + float(c) ** 2))))
    r2 = NF32(-(c * r1))
    w[L - 2, 0] = NF32(1.0)
    nr[L - 2, 0] = r1
    w[L - 1, 0] = NF32(1.0)
    nr[L - 1, 0] = r2
    return w, nr


def _make_in_maps(weights, nr):
    weights = np.ascontiguousarray(weights, dtype=NF32)
    nr = np.ascontiguousarray(nr, dtype=NF32)
    pre_w, pre_nr = _core0_prepend()
    in_maps = []
    for m in range(N_CORES):
        s = m * OWN
        if m == 0:
            wm = np.concatenate([pre_w, weights[:OWN]])
            rm = np.concatenate([pre_nr, nr[:OWN]])
        else:
            wm = weights[s - L:s + OWN]
            rm = nr[s - L:s + OWN]
        in_maps.append({"w": wm, "nr": rm})
    return in_maps


def _run(in_maps, **kwargs):
    nc = _get_program()
    return run_bass_kernel_spmd(nc, in_maps, core_ids=list(range(N_CORES)), **kwargs)


def kernel(weights, next_returns):
    in_maps = _make_in_maps(weights, next_returns)
    res = _run(in_maps)
    total = np.sum(
        np.array([res.results[m]["out"][0, 0] for m in range(N_CORES)], NF32),
        dtype=NF32,
    )
    return NF32(NF32(SQETA) * total / NF32(B_TOTAL))
